# revision 1
# baseline (speedup 1.0000x reference)
"""Trainium2 Bass kernel for nn_AttentionPooler.

Computes out[b,s,p] = sum_n relu(x[b,n,s,:] @ W1 + b1) @ W2 + N*b2
for x [32, 512, 32, 64] fp32, sharded data-parallel over 8 NeuronCores
(4 batch elements per core).

The device pipeline runs in fp16 with fp32 PSUM accumulation. The host
casts x to fp16 and packs TOKEN PAIRS: adjacent tokens (2t, 2t+1) are
interleaved per w so one 4-byte word = (fp16[x[2t,w]], fp16[x[2t+1,w]]).
The on-chip transpose then operates on fp32-typed words, moving two
tokens per element: half the DMA bytes and half the DVE transpose work
vs fp32.

Per-core dataflow (per batch element b, per 2048-token / 1024-pair chunk c):
  1. 4x dma_start loads packed x into D [128, 512] (fp32 words) laid out
     so one DVE 32x32 block-transpose yields xT with w on partitions:
     partitions 0-63  = w(0..63) of pair-tokens, H=0 half
     partitions 64-127 = same for H=1 half (2-stacked for full PE use)
  2. nc.vector.transpose  D -> xT (fp32 words; xT tile is fp16 [128,1024])
  3. matmul z = blkdiag(W1,W1).T @ xT_fp16   2x N=512 (PE, fp16, fp32 PSUM)
  4. h = relu(z + [b1;b1])  one ACT op over [128, 1024] PSUM -> SBUF fp16
  5. matmul y_acc += [W2;W2].T @ h  2x N=512, PSUM-accumulate over all
     chunks/halves == the ragged-N reduction
Epilogue per b: copy y_acc [64,512] to SBUF, DMA to DRAM.
Host: group the 512 columns by s = (2*((j//2)%32) + j%2) % 32, sum,
transpose, + N*b2.
"""

import sys

if "/opt/trn_rl_repo" not in sys.path:
    sys.path.insert(0, "/opt/trn_rl_repo")

from contextlib import ExitStack

import numpy as np

import concourse.bass as bass
import concourse.tile as tile
from concourse import bacc, mybir
from concourse.bass_utils import run_bass_kernel_spmd

B, N_ITEMS, S, W, P_OUT = 32, 512, 32, 64, 64
NCORES = 8
B_LOC = B // NCORES          # 4 batch elements per core
CHUNKS = 8                   # chunks per batch element (1024 pairs each)

F32 = mybir.dt.float32
F16 = mybir.dt.float16
RELU = mybir.ActivationFunctionType.Relu
COPY = mybir.ActivationFunctionType.Copy


def build_nc():
    # Bacc (not plain Bass): its finalize() runs generate_event_semaphores(),
    # which legalizes the TRN2 one-sync-wait-per-instruction constraint by
    # hoisting extra waits onto InstEventSemaphore instructions.
    nc = bacc.Bacc(None, target_bir_lowering=False)
    # x packed pairs, factored [b, m, H, u, J, v] over fp32 words:
    # pair = 1024*(m//16) + 64*(m%16) + 32H + u, w = 32J + v.
    # The interleaved H placement makes m = 16c + g dense in DRAM, so one
    # dma_start per (b, H, J) covers all chunks (dma_start issue costs
    # ~1.2us of sequencer time each — instruction count dominates).
    x = nc.declare_dram_parameter(
        "x", [B_LOC, 128, 2, 32, 2, 32], F32, isOutput=False
    )
    w1blk = nc.declare_dram_parameter("w1blk", [128, 128], F16, isOutput=False)
    w2stk = nc.declare_dram_parameter("w2stk", [128, 64], F16, isOutput=False)
    b1stk = nc.declare_dram_parameter("b1stk", [128, 1], F32, isOutput=False)
    yout = nc.declare_dram_parameter("y", [B_LOC, 64, 512], F32, isOutput=True)

    with ExitStack() as ctx:
        tc = ctx.enter_context(tile.TileContext(nc))
        consts = ctx.enter_context(tc.tile_pool(name="consts", bufs=1))
        dpool = ctx.enter_context(tc.tile_pool(name="dpool", bufs=2))
        xtpool = ctx.enter_context(tc.tile_pool(name="xtpool", bufs=2))
        hpool = ctx.enter_context(tc.tile_pool(name="hpool", bufs=3))
        opool = ctx.enter_context(tc.tile_pool(name="opool", bufs=2))
        zpool = ctx.enter_context(
            tc.tile_pool(name="zpool", bufs=2, space=bass.MemorySpace.PSUM)
        )
        ypool = ctx.enter_context(
            tc.tile_pool(name="ypool", bufs=2, space=bass.MemorySpace.PSUM)
        )

        sw1 = consts.tile([128, 128], F16)
        nc.sync.dma_start(out=sw1[:, :], in_=w1blk[:, :])
        sw2 = consts.tile([128, 64], F16)
        nc.sync.dma_start(out=sw2[:, :], in_=w2stk[:, :])
        sb1 = consts.tile([128, 1], F32)
        nc.sync.dma_start(out=sb1[:, :], in_=b1stk[:, :])

        for b in range(B_LOC):
            y_acc = ypool.tile([64, 512], F32)
            d = dpool.tile([128, 4096], F32)
            for hh in range(2):
                for jj in range(2):
                    src = x[b, :, hh, :, jj, :].rearrange("m u v -> u m v")
                    p0 = 64 * hh + 32 * jj
                    nc.sync.dma_start(out=d[p0 : p0 + 32, :], in_=src)
            xt = xtpool.tile([128, 8192], F16)
            nc.vector.transpose(out=xt[:, :].bitcast(F32), in_=d[:, :])
            for c in range(CHUNKS):
                z = zpool.tile([128, 1024], F32)
                for i in range(2):
                    nc.tensor.matmul(
                        z[:, 512 * i : 512 * i + 512],
                        sw1[:, :],
                        xt[:, 1024 * c + 512 * i : 1024 * c + 512 * i + 512],
                        start=True,
                        stop=True,
                    )
                h = hpool.tile([128, 1024], F16)
                if c < 6:
                    # relu + bias on ACT
                    nc.scalar.activation(
                        h[:, :], z[:, :], RELU, bias=sb1[:, 0:1], scale=1.0
                    )
                else:
                    # relu + bias on DVE: (z + b1) max 0 — balances ACT vs DVE
                    nc.vector.tensor_scalar(
                        h[:, :],
                        z[:, :],
                        sb1[:, 0:1],
                        0.0,
                        mybir.AluOpType.add,
                        mybir.AluOpType.max,
                    )
                for i in range(2):
                    nc.tensor.matmul(
                        y_acc[:, :],
                        sw2[:, :],
                        h[:, 512 * i : 512 * i + 512],
                        start=(c == 0 and i == 0),
                        stop=(c == CHUNKS - 1 and i == 1),
                    )
            o = opool.tile([64, 512], F32)
            nc.scalar.activation(o[:, :], y_acc[:, :], COPY, scale=1.0)
            nc.sync.dma_start(out=yout[b, :, :], in_=o[:, :])
    nc.finalize()
    return nc


def _pack_x(inputs):
    x16 = np.asarray(inputs, dtype=np.float16)       # [B, N, S, W]
    pairs = x16.reshape(NCORES, B_LOC, CHUNKS * 1024, 2, W)
    pairs = np.ascontiguousarray(pairs.swapaxes(3, 4))  # [8, 4, pairs, W, 2] fp16
    words = pairs.view(np.float32).reshape(NCORES, B_LOC, CHUNKS * 1024, W)
    # factor pair index: pt = 1024c + 64g + 32H + u ; w = 32J + v.
    # C-order reshape [c, g, H, u] == pt-major, then merge m = 16c + g.
    return np.ascontiguousarray(
        words.reshape(NCORES, B_LOC, 128, 2, 32, 2, 32)
    )


def prep_weights(W1, b1, W2):
    w1blk = np.zeros((128, 128), np.float16)
    w1blk[:64, :64] = np.asarray(W1, np.float16)
    w1blk[64:, 64:] = np.asarray(W1, np.float16)
    w2stk = np.ascontiguousarray(
        np.concatenate([W2, W2], axis=0), dtype=np.float16
    )
    b1stk = np.ascontiguousarray(
        np.concatenate([b1, b1]).reshape(128, 1), dtype=np.float32
    )
    return w1blk, w2stk, b1stk


_SMAP = (2 * ((np.arange(512) // 2) % 32) + (np.arange(512) % 2)) % 32
_SORT = np.argsort(_SMAP, kind="stable")


def postprocess(y, b2):
    # y: [NCORES, B_LOC, 64, 512] -> out [B, S, P]
    y = np.asarray(y, dtype=np.float32).reshape(B, 64, 512)
    y = y[:, :, _SORT].reshape(B, 64, 32, 16)        # cols grouped by s
    out_t = y.sum(axis=3, dtype=np.float32)          # [B, p, s]
    out = out_t.transpose(0, 2, 1) + np.float32(N_ITEMS) * np.asarray(
        b2, dtype=np.float32
    )
    return np.ascontiguousarray(out, dtype=np.float32)


def kernel(inputs, W1, b1, W2, b2, _trace=False):
    xw = _pack_x(inputs)
    w1blk, w2stk, b1stk = prep_weights(W1, b1, W2)
    nc = build_nc()
    in_maps = [
        {
            "x": np.ascontiguousarray(xw[i]),
            "w1blk": w1blk,
            "w2stk": w2stk,
            "b1stk": b1stk,
        }
        for i in range(NCORES)
    ]
    res = run_bass_kernel_spmd(nc, in_maps, list(range(NCORES)), trace=_trace)
    y = np.stack([res.results[i]["y"] for i in range(NCORES)])
    out = postprocess(y, b2)
    if _trace:
        return out, res
    return out



# revision 5
# speedup vs baseline: 2.7626x; 2.7626x over previous
"""Trainium2 Bass kernel for nn_AttentionPooler (v2).

Computes out[b,s,p] = sum_n relu(x[b,n,s,:] @ W1 + b1) @ W2 + N*b2
for x [32, 512, 32, 64] fp32, data-parallel over 8 NeuronCores
(4 batch elements per core, 65536 tokens per core).

Design (per core):
  - Host packs x as fp8 e3m4 in a PRE-TRANSPOSED layout [b, 128, 8192]:
    partition p = w + 64*(n&1), column c = (n>>1)*32 + s. Each partition's
    row is 8KB contiguous in DRAM -> 4 big full-bandwidth DMAs, no on-chip
    transpose at all (the old kernel burned ~66k 128B DMA descriptors).
  - W1 stays bf16 as blkdiag(W1,W1) [128,128]; mixed-dtype matmul
    (bf16 lhsT x fp8 rhs) computes z for 2 tokens per column.
    64 matmuls of 512 cols -> z in PSUM fp32.
  - relu+bias runs split across ACT and DVE (the PSUM->SBUF move is the
    hard floor: 1 elem/lane/cycle on each engine):
      * ACT share (9 tiles/b): activation(Relu) writing fp8 e4m3 in a
        DoubleRow-interleaved layout; consumed by fp8 DoubleRow W2
        matmuls (4 tokens summed per output column on the PE).
      * DVE share (7 tiles/b): scalar_tensor_tensor chains
        h_acc = relu(z) + h_acc in fp32 SBUF - relu AND the ragged-N
        reduction in one 1x pass per tile; chain end is cast to bf16 and
        fed to a bf16 W2 matmul.
  - Both W2 weight sets use the SAME quantized values W2q = e4m3(W2) and
    carry an identity passthrough block in columns 64..127, so PSUM
    accumulates y = h@W2q in partitions 0..63 and hsum = sum_n h in
    partitions 64..127. The host applies the exact correction
    y += hsum @ (W2 - W2q), making W2 effectively full precision.
  - y accumulates in PSUM [128, 256] per b (col = g*32+s, g in 0..7);
    host folds g, applies corrections, adds N*b2.
"""

import sys

if "/opt/trn_rl_repo" not in sys.path:
    sys.path.insert(0, "/opt/trn_rl_repo")

from contextlib import ExitStack

import ml_dtypes
import numpy as np

import concourse.bass as bass
import concourse.tile as tile
from concourse import bacc, mybir
from concourse.bass_utils import run_bass_kernel_spmd

B, N_ITEMS, S, W, P_OUT = 32, 512, 32, 64, 64
NCORES = 8
B_LOC = B // NCORES          # 4 batch elements per core
TILES_PER_B = 16             # z tiles of 512 cols per batch element
ACT_TILES = 9                # leading tiles -> ACT/fp8-DoubleRow path
DVE_TILES = TILES_PER_B - ACT_TILES   # trailing tiles -> DVE chain path
N_STT = DVE_TILES - 1        # chain steps that miss the +b1 term

F32 = mybir.dt.float32
BF16 = mybir.dt.bfloat16
F8E3 = mybir.dt.float8e3
F8E4 = mybir.dt.float8e4
RELU = mybir.ActivationFunctionType.Relu
COPY = mybir.ActivationFunctionType.Copy
ADD = mybir.AluOpType.add
MAX = mybir.AluOpType.max
DR = mybir.MatmulPerfMode.DoubleRow

NP_E3 = ml_dtypes.float8_e3m4
NP_E4 = ml_dtypes.float8_e4m3
NP_BF16 = ml_dtypes.bfloat16


def build_nc():
    nc = bacc.Bacc(None, target_bir_lowering=False)
    x = nc.declare_dram_parameter("x", [B_LOC, 128, 8192], F8E3, isOutput=False)
    w1blk = nc.declare_dram_parameter("w1blk", [128, 128], BF16, isOutput=False)
    w2dr = nc.declare_dram_parameter("w2dr", [128, 256], F8E4, isOutput=False)
    w2bf = nc.declare_dram_parameter("w2bf", [128, 128], BF16, isOutput=False)
    b1stk = nc.declare_dram_parameter("b1stk", [128, 1], F32, isOutput=False)
    b1neg = nc.declare_dram_parameter("b1neg", [128, 1], F32, isOutput=False)
    yout = nc.declare_dram_parameter("y", [B_LOC, 128, 256], F32, isOutput=True)

    with ExitStack() as ctx:
        tc = ctx.enter_context(tile.TileContext(nc))
        consts = ctx.enter_context(tc.tile_pool(name="consts", bufs=1))
        xpool = ctx.enter_context(tc.tile_pool(name="xpool", bufs=B_LOC))
        hdrp = ctx.enter_context(tc.tile_pool(name="hdrp", bufs=3))
        haccp = ctx.enter_context(tc.tile_pool(name="haccp", bufs=2))
        hbfp = ctx.enter_context(tc.tile_pool(name="hbfp", bufs=2))
        opool = ctx.enter_context(tc.tile_pool(name="opool", bufs=2))
        zap = ctx.enter_context(
            tc.tile_pool(name="zap", bufs=2, space=bass.MemorySpace.PSUM)
        )
        zdp = ctx.enter_context(
            tc.tile_pool(name="zdp", bufs=2, space=bass.MemorySpace.PSUM)
        )
        ypool = ctx.enter_context(
            tc.tile_pool(name="ypool", bufs=1, space=bass.MemorySpace.PSUM)
        )

        sw1 = consts.tile([128, 128], BF16)
        nc.sync.dma_start(out=sw1[:, :], in_=w1blk[:, :])
        sw2dr = consts.tile([128, 256], F8E4)
        nc.sync.dma_start(out=sw2dr[:, :], in_=w2dr[:, :])
        sw2bf = consts.tile([128, 128], BF16)
        nc.sync.dma_start(out=sw2bf[:, :], in_=w2bf[:, :])
        sb1 = consts.tile([128, 1], F32)
        nc.sync.dma_start(out=sb1[:, :], in_=b1stk[:, :])
        sb1n = consts.tile([128, 1], F32)
        nc.sync.dma_start(out=sb1n[:, :], in_=b1neg[:, :])

        xs = []
        for b in range(B_LOC):
            xt = xpool.tile([128, 8192], F8E3, name=f"x_{b}")
            nc.sync.dma_start(out=xt[:, :], in_=x[b, :, :])
            xs.append(xt)

        # DR weight view [K=128, 2, M=128] (slot-major layout, slot stride
        # 128 bytes). Both slots carry identical weights: the slot dim is
        # the virtual-contraction pair being summed.
        w2dr_ap = sw2dr[:, :].rearrange("p (t m) -> p t m", t=2)

        y_all = ypool.tile([128, 1024], F32, name="y_all")
        o_all = opool.tile([128, 1024], F32, name="o_all")

        for b in range(B_LOC):
            y = y_all[:, 256 * b : 256 * b + 256]
            first_mm = [True]

            def y_mm_flags(last=False):
                st = first_mm[0]
                first_mm[0] = False
                return dict(start=st, stop=last)

            # --- ACT / DoubleRow share ---
            groups = [(0, 2), (2, 2), (4, 2), (6, 2), (8, 1)]
            for t0, ntile in groups:
                L = 512 * ntile
                z = zap.tile([128, 1024], F32, name="z_act")
                for i in range(ntile):
                    c0 = (t0 + i) * 512
                    nc.tensor.matmul(
                        z[:, 512 * i : 512 * i + 512],
                        sw1[:, :],
                        xs[b][:, c0 : c0 + 512],
                        start=True,
                        stop=True,
                    )
                hdr = hdrp.tile([128, 1024], F8E4, name="hdr")
                # relu(z*1 + b1) -> fp8, written DR-interleaved:
                # src col c = m2*64 + t*32 + s ; dst byte = m2*64 + s*2 + t
                m2 = L // 64
                nc.scalar.activation(
                    hdr[:, :L].rearrange("p (m2 s t) -> p m2 t s", m2=m2, s=32, t=2),
                    z[:, :L].rearrange("p (m2 t s) -> p m2 t s", m2=m2, t=2, s=32),
                    RELU,
                    bias=sb1[:, 0:1],
                    scale=1.0,
                )
                for i in range(ntile):
                    rhs = hdr[:, 512 * i : 512 * i + 512].rearrange(
                        "p (m2 s t) -> p t m2 s", m2=8, s=32, t=2
                    )
                    nc.tensor.matmul(
                        y[:, :],
                        w2dr_ap,
                        rhs,
                        perf_mode=DR,
                        **y_mm_flags(),
                    )

            # --- DVE chain share ---
            hacc = None
            for j in range(DVE_TILES):
                c0 = (ACT_TILES + j) * 512
                z = zdp.tile([128, 512], F32, name="z_dve")
                nc.tensor.matmul(
                    z[:, :], sw1[:, :], xs[b][:, c0 : c0 + 512],
                    start=True, stop=True,
                )
                hnew = haccp.tile([128, 512], F32, name="hacc")
                if j == 0:
                    # h = max(z + b1, 0)
                    nc.vector.tensor_scalar(
                        hnew[:, :], z[:, :], sb1[:, 0:1], 0.0, ADD, MAX
                    )
                else:
                    # h = max(z, -b1) + hacc  (= relu(z+b1) - b1 + hacc)
                    nc.vector.scalar_tensor_tensor(
                        hnew[:, :], z[:, :], sb1n[:, 0:1], hacc[:, :], MAX, ADD
                    )
                hacc = hnew
            hbf = hbfp.tile([128, 512], BF16, name="hbf")
            nc.vector.tensor_scalar(hbf[:, :], hacc[:, :], 0.0, None, ADD)
            for i in range(2):
                nc.tensor.matmul(
                    y[:, :],
                    sw2bf[:, :],
                    hbf[:, 256 * i : 256 * i + 256],
                    **y_mm_flags(last=(i == 1)),
                )

            o = o_all[:, 256 * b : 256 * b + 256]
            nc.scalar.activation(o, y, COPY, scale=1.0)
            nc.sync.dma_start(out=yout[b, :, :], in_=o)
    nc.finalize()
    return nc


def _pack_x(inputs):
    x8 = np.asarray(inputs, dtype=np.float32).astype(NP_E3)
    # [core, b, m, parity, s, w] -> [core, b, (parity w), (m s)]
    x8 = x8.reshape(NCORES, B_LOC, 256, 2, S, W)
    x8 = x8.transpose(0, 1, 3, 5, 2, 4)
    return np.ascontiguousarray(x8).reshape(NCORES, B_LOC, 128, 8192)


def prep_weights(W1, b1, W2):
    w1blk = np.zeros((128, 128), NP_BF16)
    w1blk[:64, :64] = np.asarray(W1, np.float32).astype(NP_BF16)
    w1blk[64:, 64:] = w1blk[:64, :64]

    W2q8 = np.asarray(W2, np.float32).astype(NP_E4)           # [64, 64]
    W2q = W2q8.astype(np.float32)
    ident = np.eye(64, dtype=np.float32)
    w2half = np.concatenate([W2q, ident], axis=1)             # [64, 128]
    w2full = np.concatenate([w2half, w2half], axis=0)         # [128, 128]
    w2dr = np.concatenate([w2full, w2full], axis=1).astype(NP_E4)  # [128, 256]
    w2bf = w2full.astype(NP_BF16)

    b1f = np.asarray(b1, np.float32)
    b1stk = np.concatenate([b1f, b1f]).reshape(128, 1).astype(np.float32)
    return w1blk, w2dr, w2bf, b1stk, -b1stk, W2q


def postprocess(y, W2, W2q, b1, b2):
    # y: [NCORES, B_LOC, 128, 256] fp32
    y = np.asarray(y, np.float32).reshape(NCORES, B_LOC, 128, 8, 32)
    yq = y[:, :, 0:64].sum(axis=3)        # [core, b, p, s]
    hs = y[:, :, 64:128].sum(axis=3)      # [core, b, k, s]
    b1f = np.asarray(b1, np.float32)
    # DVE chain misses +b1 on N_STT of its adds: per y-col the bf16-W2
    # matmul folds 2 rhs cols x 2 partition halves -> 4*N_STT*b1 deficit.
    deficit = (4.0 * N_STT) * b1f                                  # [64]
    hs_true = hs + deficit[None, None, :, None]
    dW2 = np.asarray(W2, np.float32) - W2q
    out = (
        yq
        + np.einsum("cbks,kp->cbps", hs_true, W2q + dW2, dtype=np.float32)
        - np.einsum("cbks,kp->cbps", hs, W2q, dtype=np.float32)
    )
    # out = yq + deficit@W2q + hs_true@dW2  (expanded to avoid cancellation
    # confusion: yq already holds hs@W2q content? no - yq is h@W2q; the two
    # einsums add hs_true@(W2q+dW2) - hs@W2q = deficit@W2q + hs_true@dW2).
    out = out + np.float32(N_ITEMS) * np.asarray(b2, np.float32)[None, None, :, None]
    out = out.reshape(B, 64, 32).transpose(0, 2, 1)   # [B, S, P]
    return np.ascontiguousarray(out, np.float32)


def kernel(inputs, W1, b1, W2, b2, _trace=False):
    xw = _pack_x(inputs)
    w1blk, w2dr, w2bf, b1stk, b1neg, W2q = prep_weights(W1, b1, W2)
    nc = build_nc()
    in_maps = [
        {
            "x": np.ascontiguousarray(xw[i]),
            "w1blk": w1blk,
            "w2dr": w2dr,
            "w2bf": w2bf,
            "b1stk": b1stk,
            "b1neg": b1neg,
        }
        for i in range(NCORES)
    ]
    res = run_bass_kernel_spmd(nc, in_maps, list(range(NCORES)), trace=_trace)
    y = np.stack([res.results[i]["y"] for i in range(NCORES)])
    out = postprocess(y, W2, W2q, b1, b2)
    if _trace:
        return out, res
    return out
